# revision 14
# baseline (speedup 1.0000x reference)
"""Distributed RoPE causal attention for Trainium2 (8 NeuronCores), v2.

Sharding: each core owns 2 heads (tensor-parallel over 16 heads) for both
batches. One 8-core AllToAll per lq-chunk redistributes per-head outputs so
every core gets a 64-token slice (per batch) of ALL heads, then runs 1/8 of
the output projection for that chunk.

v2: software-pipelined emission — the next step's projection matmuls and
RoPE ops are interleaved between attention t-iterations so the PE/DVE FIFOs
always have ready work during the ACT-bound exp chain; RoPE is staged
through one bf16 SBUF copy (2x/4x DVE modes); softmax denominators ride at
partition 0 of the AV psum (direct PSUM reciprocal).
"""
import sys

sys.path.insert(0, "/opt/trn_rl_repo")

import numpy as np
import ml_dtypes

import concourse.bass as bass
import concourse.tile as tile
from concourse import bacc, mybir
from concourse import bass_utils

B, L, H, D = 2, 2048, 16, 64
HID = H * D
NC = 8
CH = 512          # lq chunk width
NCH = L // CH     # 4 chunks per batch
NT = L // 128     # 16 k-tiles of 128 per batch
TB = CH // NC     # 64-token slice each core owns per (chunk, batch)
F32 = mybir.dt.float32
BF16 = mybir.dt.bfloat16
AF = mybir.ActivationFunctionType
ALU = mybir.AluOpType

_CACHE = {}


def build():
    nc = bacc.Bacc("TRN2", target_bir_lowering=False, debug=False, num_devices=NC)

    xT_e = nc.dram_tensor("xT", [B, HID, L], BF16, kind="ExternalInput")
    wq_e = nc.dram_tensor("wq", [HID, 128], BF16, kind="ExternalInput")
    wk_e = nc.dram_tensor("wk", [HID, 128], BF16, kind="ExternalInput")
    wv_e = nc.dram_tensor("wv", [HID, 128], BF16, kind="ExternalInput")
    wo_e = nc.dram_tensor("wo", [HID, HID], BF16, kind="ExternalInput")
    cos_e = nc.dram_tensor("cosd", [128, 2, L], BF16, kind="ExternalInput")
    sin_e = nc.dram_tensor("sind", [128, 2, L], BF16, kind="ExternalInput")
    tri_e = nc.dram_tensor("tri", [128, 128], BF16, kind="ExternalInput")
    out_e = nc.dram_tensor("out", [NCH, 2 * TB, HID], F32, kind="ExternalOutput")

    xT_r = xT_e.ap().rearrange("b (kt p) l -> b p kt l", p=128)

    with tile.TileContext(nc) as tc:
        with tc.tile_pool(name="const", bufs=1) as cpool, \
             tc.tile_pool(name="persist", bufs=1) as ppool, \
             tc.tile_pool(name="xin", bufs=4) as xpool, \
             tc.tile_pool(name="tmp", bufs=4) as tpool, \
             tc.tile_pool(name="ptp", bufs=8) as ptpool, \
             tc.tile_pool(name="osb", bufs=3) as opool, \
             tc.tile_pool(name="fin", bufs=2) as fpool, \
             tc.tile_pool(name="ps_proj", bufs=1, space="PSUM") as ps_proj, \
             tc.tile_pool(name="ps_s", bufs=2, space="PSUM") as ps_s, \
             tc.tile_pool(name="ps_o", bufs=2, space="PSUM") as ps_o, \
             tc.tile_pool(name="dram", bufs=1, space="DRAM") as dpool:

            wq_sb = cpool.tile([128, 8, 128], BF16)
            wk_sb = cpool.tile([128, 8, 128], BF16)
            wv_sb = cpool.tile([128, 8, 128], BF16)
            wo_sb = cpool.tile([128, 8, HID], BF16)
            nc.sync.dma_start(wq_sb[:], wq_e.ap().rearrange("(kt p) m -> p kt m", p=128))
            nc.sync.dma_start(wk_sb[:], wk_e.ap().rearrange("(kt p) m -> p kt m", p=128))
            nc.sync.dma_start(wv_sb[:], wv_e.ap().rearrange("(kt p) m -> p kt m", p=128))
            cosd_sb = cpool.tile([128, 2, L], BF16)
            sind_sb = cpool.tile([128, 2, L], BF16)
            tri2_sb = cpool.tile([128, 2, 128], BF16)
            nc.sync.dma_start(tri2_sb[:, 0], tri_e[:, :])
            nc.sync.dma_start(tri2_sb[:, 1], tri_e[:, :])

            # persistent per-core tensors: q and k fused on one axis so RoPE
            # runs as single [128, 2, CH] DVE ops
            qkT_sb = ppool.tile([128, 2, B, L], BF16)   # [64*hl+d, q|k, b, l]
            v_sb = ppool.tile([128, B, NT, 130], BF16)  # [lk%128, b, t, 65*hl + (ones|d)]

            # chunks 0-2 gather both batches in one A2A; the last chunk goes
            # per batch so most of its output projection hides under the
            # final t-loop
            a2a_in = [dpool.tile([NC, 128, 2 * TB], BF16, name=f"a2ain{i}",
                                 tag=f"a2ain{i}") for i in range(NCH - 1)]
            a2a_out = [dpool.tile([NC, 128, 2 * TB], BF16, name=f"a2aout{i}",
                                  tag=f"a2aout{i}") for i in range(NCH - 1)]
            a2a_in3 = [dpool.tile([NC, 128, TB], BF16, name=f"a2ain3{b}",
                                  tag=f"a2ain3{b}") for b in range(B)]
            a2a_out3 = [dpool.tile([NC, 128, TB], BF16, name=f"a2aout3{b}",
                                   tag=f"a2aout3{b}") for b in range(B)]
            # dummy warm-up collective: the first collective of a NEFF pays
            # ~30us of ncfw/ENCD setup; burn it at t=0 under the input loads.
            # Standalone dram tensors (not dpool tiles) so pool-level dep
            # tracking can't chain it behind real compute.
            wu_in = nc.dram_tensor("wuin", [NC, 64], BF16, kind="Internal").ap()
            wu_out = nc.dram_tensor("wuout", [NC, 64], BF16, kind="Internal").ap()
            nc.gpsimd.collective_compute(
                "AllToAll", ALU.bypass, replica_groups=[list(range(NC))],
                ins=[wu_in.opt()], outs=[wu_out.opt()])

            steps = [(j, b) for j in range(NCH) for b in range(B)]

            def emit_loads(i):
                j, b = steps[i]
                ls = j * CH
                xc = xpool.tile([128, 8, CH], BF16, name="xc")
                for kt in range(8):
                    nc.sync.dma_start(xc[:, kt], xT_r[b][:, kt, ls:ls + CH])
                if b == 0:
                    nc.sync.dma_start(cosd_sb[:, :, ls:ls + CH],
                                      cos_e[:, :, ls:ls + CH])
                    nc.sync.dma_start(sind_sb[:, :, ls:ls + CH],
                                      sin_e[:, :, ls:ls + CH])
                if i == 1:
                    nc.sync.dma_start(
                        wo_sb[:], wo_e.ap().rearrange("(kt p) m -> p kt m", p=128))
                return xc

            def proj_closures(i, xc):
                """Fine-grained closures for step i's QKV projections + RoPE,
                to be emitted interleaved into the previous t-loop."""
                j, b = steps[i]
                ls = j * CH
                box = {}

                def c_q():
                    box["pp"] = ps_proj.tile([128, 2, CH], F32, tag="proj", name="pp")
                    for kt in range(8):
                        nc.tensor.matmul(box["pp"][:, 0], wq_sb[:, kt], xc[:, kt],
                                         start=(kt == 0), stop=(kt == 7))

                def c_k():
                    for kt in range(8):
                        nc.tensor.matmul(box["pp"][:, 1], wk_sb[:, kt], xc[:, kt],
                                         start=(kt == 0), stop=(kt == 7))

                def c_stage():
                    # single PSUM reader: everything downstream is bf16 SBUF
                    box["ppb"] = tpool.tile([128, 2, CH], BF16, name="ppb")
                    nc.vector.tensor_copy(box["ppb"][:], box["pp"][:])

                def c_rot():
                    rot = box["rot"] = tpool.tile([128, 2, CH], BF16, name="rot")
                    for seg in range(4):
                        po_, pi_ = 32 * seg, 32 * (seg ^ 1)
                        nc.vector.tensor_copy(rot[po_:po_ + 32], box["ppb"][pi_:pi_ + 32])

                def c_mul():
                    box["t1"] = tpool.tile([128, 2, CH], BF16, name="t1")
                    nc.vector.tensor_mul(box["t1"][:], box["ppb"][:],
                                         cosd_sb[:, :, ls:ls + CH])
                    box["t2"] = tpool.tile([128, 2, CH], BF16, name="t2")
                    nc.vector.tensor_mul(box["t2"][:], box["rot"][:],
                                         sind_sb[:, :, ls:ls + CH])

                def c_add():
                    nc.vector.tensor_add(qkT_sb[:, :, b, ls:ls + CH],
                                         box["t1"][:], box["t2"][:])

                def c_v0():
                    box["pv"] = ps_proj.tile([128, 4, 128], F32, tag="proj", name="pv")
                    for tl in (0, 1):
                        for kt in range(8):
                            nc.tensor.matmul(box["pv"][:, tl],
                                             xc[:, kt, 128 * tl:128 * tl + 128],
                                             wv_sb[:, kt], start=(kt == 0),
                                             stop=(kt == 7), skip_group_check=True)

                def c_v1():
                    for tl in (2, 3):
                        for kt in range(8):
                            nc.tensor.matmul(box["pv"][:, tl],
                                             xc[:, kt, 128 * tl:128 * tl + 128],
                                             wv_sb[:, kt], start=(kt == 0),
                                             stop=(kt == 7), skip_group_check=True)

                def c_vcopy():
                    nc.vector.memset(v_sb[:, b, 4 * j:4 * j + 4, 64::65], 1.0)
                    vv = v_sb.rearrange("p b t (hl e) -> p b t hl e", e=65)
                    nc.vector.tensor_copy(
                        vv[:, b, 4 * j:4 * j + 4, :, 0:64],
                        box["pv"].rearrange("p tl (hl d) -> p tl hl d", d=64))

                return [c_q, c_k, c_stage, c_rot, c_mul, c_add, c_v0, c_v1, c_vcopy]

            def outproj_closures(j):
                """1/8 of chunk j's output projection (this core's 2x64
                tokens, full 1024 out cols)."""
                box = {}

                def c0():
                    box["oj"] = fpool.tile([128, 8, 2 * TB], BF16, name="oj")
                    nc.sync.dma_start(box["oj"][:],
                                      a2a_out[j].rearrange("s p t -> p s t"))
                    box["py"] = ps_proj.tile([128, 2, 512], F32, tag="proj", name="py")
                    for kt in range(8):
                        nc.tensor.matmul(box["py"][:, 0], box["oj"][:, kt],
                                         wo_sb[:, kt, 0:512],
                                         start=(kt == 0), stop=(kt == 7))

                def c1():
                    for kt in range(8):
                        nc.tensor.matmul(box["py"][:, 1], box["oj"][:, kt],
                                         wo_sb[:, kt, 512:1024],
                                         start=(kt == 0), stop=(kt == 7))
                    ysb = fpool.tile([128, 2, 512], F32, name="ysb")
                    nc.vector.tensor_copy(ysb[:], box["py"][:])
                    nc.sync.dma_start(out_e[j], ysb.rearrange("p n m -> p (n m)"))

                return [c0, c1]

            def outproj3_closures(b):
                """Last chunk, one batch: 64-token stationary."""
                box = {}

                def c0():
                    box["oj"] = fpool.tile([128, 8, TB], BF16, name="oj3")
                    nc.sync.dma_start(box["oj"][:],
                                      a2a_out3[b].rearrange("s p t -> p s t"))
                    box["py"] = ps_proj.tile([64, 2, 512], F32, tag="proj", name="py3")
                    for kt in range(8):
                        nc.tensor.matmul(box["py"][:, 0], box["oj"][:, kt],
                                         wo_sb[:, kt, 0:512],
                                         start=(kt == 0), stop=(kt == 7))

                def c1():
                    for kt in range(8):
                        nc.tensor.matmul(box["py"][:, 1], box["oj"][:, kt],
                                         wo_sb[:, kt, 512:1024],
                                         start=(kt == 0), stop=(kt == 7))
                    ysb = fpool.tile([64, 2, 512], F32, name="ysb3")
                    nc.vector.tensor_copy(ysb[:], box["py"][:])
                    nc.sync.dma_start(out_e[NCH - 1, b * TB:(b + 1) * TB, :],
                                      ysb.rearrange("p n m -> p (n m)"))

                return [c0, c1]

            def emit_attention(i, pend):
                """Step i's attention t-loop; `pend` closures are drained
                between t-iterations (after the mask, before the AV matmuls)
                so the PE/DVE FIFOs have dep-free work during the exp wait."""
                j, b = steps[i]
                ls = j * CH
                niter = 4 * j + 4
                n0 = len(pend)
                emitted = 0
                po_t = [ps_o.tile([65, CH], F32, tag="o", name=f"po{hl}")
                        for hl in range(2)]
                for t in range(niter):
                    tl = t - 4 * j
                    o0 = 128 * tl if tl > 0 else 0   # skip fully-masked cols
                    pss = ps_s.tile([128, 2, CH], F32, tag="s", name="pss")
                    for hl in range(2):
                        hp = 64 * hl
                        nc.tensor.matmul(
                            pss[:, hl, o0:CH],
                            qkT_sb[hp:hp + 64, 1, b, 128 * t:128 * t + 128],
                            qkT_sb[hp:hp + 64, 0, b, ls + o0:ls + CH],
                            start=True, stop=True)
                    pt = ptpool.tile([128, 2, CH], BF16, name="pt")
                    nc.scalar.activation(pt[:, :, o0:CH], pss[:, :, o0:CH],
                                         AF.Exp, scale=0.125)
                    if tl >= 0:
                        nc.vector.tensor_mul(pt[:, :, o0:o0 + 128],
                                             pt[:, :, o0:o0 + 128], tri2_sb[:])
                    # drain pipelined closures: spread linearly over the loop
                    target = (t + 1) * n0 // niter
                    while emitted < target:
                        pend[emitted]()
                        emitted += 1
                    for hl in range(2):
                        nc.tensor.matmul(po_t[hl][:, o0:CH],
                                         v_sb[:, b, t, 65 * hl:65 * hl + 65],
                                         pt[:, hl, o0:CH], start=(t == 0),
                                         stop=(t == niter - 1),
                                         skip_group_check=True)
                while emitted < n0:
                    pend[emitted]()
                    emitted += 1
                return po_t

            def emit_epilogue(i, po_t):
                j, b = steps[i]
                # NB: custom-DVE recip misreads PSUM at base partition 64, so
                # stage the denominator rows through SBUF, then one batched
                # reciprocal for both heads.
                dr = tpool.tile([1, 2, CH], F32, name="dr", bufs=2)
                nc.vector.tensor_copy(dr[:, 0], po_t[0][64:65, :])
                nc.vector.tensor_copy(dr[:, 1], po_t[1][64:65, :])
                den = tpool.tile([1, 2, CH], F32, name="den", bufs=2)
                nc.vector.reciprocal_approx_fast(den[:], dr[:])
                rb = tpool.tile([64, 2, CH], F32, name="rb", bufs=2)
                nc.gpsimd.partition_broadcast(rb[:], den[:])
                o_sb = opool.tile([128, CH], BF16, name="o_sb")
                for hl in range(2):
                    nc.vector.tensor_mul(o_sb[64 * hl:64 * hl + 64, :],
                                         po_t[hl][0:64, :], rb[:, hl, :])
                if j == NCH - 1:
                    nc.sync.dma_start(a2a_in3[b].rearrange("s p t -> p s t"),
                                      o_sb.rearrange("p (s t) -> p s t", s=NC))
                    nc.gpsimd.collective_compute(
                        "AllToAll", ALU.bypass,
                        replica_groups=[list(range(NC))],
                        ins=[a2a_in3[b][:].opt()],
                        outs=[a2a_out3[b][:].opt()],
                    )
                else:
                    nc.sync.dma_start(
                        a2a_in[j].rearrange("s p (b t) -> b p s t", b=B)[b],
                        o_sb.rearrange("p (s t) -> p s t", s=NC))
                    if b == B - 1:
                        nc.gpsimd.collective_compute(
                            "AllToAll", ALU.bypass,
                            replica_groups=[list(range(NC))],
                            ins=[a2a_in[j][:].opt()],
                            outs=[a2a_out[j][:].opt()],
                        )

            # ---- software-pipelined emission ----
            xc0 = emit_loads(0)
            for cl in proj_closures(0, xc0):
                cl()
            for i in range(len(steps)):
                j, b = steps[i]
                pend = []
                if i + 1 < len(steps):
                    xc_n = emit_loads(i + 1)
                    pend += proj_closures(i + 1, xc_n)
                if b == 1 and j > 0:
                    pend += outproj_closures(j - 1)
                    if j == NCH - 1:
                        pend += outproj3_closures(0)
                po_t = emit_attention(i, pend)
                emit_epilogue(i, po_t)
            for cl in outproj3_closures(1):
                cl()

    nc.compile()
    return nc


def _shards(x, Wq, Wk, Wv, Wo, cos, sin):
    bf = ml_dtypes.bfloat16
    xT = np.ascontiguousarray(x.transpose(0, 2, 1)).astype(bf)          # (B, HID, L)
    woT = np.ascontiguousarray(Wo.T).astype(bf)                          # (HID, HID)
    cosT = cos.T.astype(np.float32)                                      # (D, L)
    sinT = sin.T.astype(np.float32)
    cos2 = np.concatenate([cosT, cosT], axis=0)                          # (128, L)
    spm = np.concatenate([-sinT[:32], sinT[32:]], axis=0)                # (64, L)
    sinpm = np.concatenate([spm, spm], axis=0)                           # (128, L)
    cosd = np.ascontiguousarray(np.stack([cos2, cos2], axis=1)).astype(bf)
    sind = np.ascontiguousarray(np.stack([sinpm, sinpm], axis=1)).astype(bf)
    # lower-triangular 128x128 mask for the diagonal band
    p = np.arange(128)[:, None]
    f = np.arange(128)[None, :]
    tri = (p <= f).astype(np.float32).astype(bf)                         # (128, 128)

    in_maps = []
    for c in range(NC):
        rows = slice(2 * c * 64, (2 * c + 2) * 64)
        in_maps.append({
            "xT": xT,
            "wq": np.ascontiguousarray(Wq[rows].T).astype(bf),
            "wk": np.ascontiguousarray(Wk[rows].T).astype(bf),
            "wv": np.ascontiguousarray(Wv[rows].T).astype(bf),
            "wo": woT,
            "cosd": cosd,
            "sind": sind,
            "tri": tri,
        })
    return in_maps


def kernel(x, Wq, Wk, Wv, Wo, cos, sin, trace=False):
    x = np.asarray(x, dtype=np.float32)
    Wq = np.asarray(Wq, dtype=np.float32)
    Wk = np.asarray(Wk, dtype=np.float32)
    Wv = np.asarray(Wv, dtype=np.float32)
    Wo = np.asarray(Wo, dtype=np.float32)
    cos = np.asarray(cos, dtype=np.float32)
    sin = np.asarray(sin, dtype=np.float32)

    if "nc" not in _CACHE:
        _CACHE["nc"] = build()
    nc = _CACHE["nc"]

    in_maps = _shards(x, Wq, Wk, Wv, Wo, cos, sin)
    kw = {}
    if trace and _CACHE.get("trace_all_cores"):
        kw["trace_cores"] = list(range(NC))
    res = bass_utils.run_bass_kernel_spmd(
        nc, in_maps, core_ids=list(range(NC)), trace=trace, **kw)
    _CACHE["last_result"] = res

    y = np.empty((B, L, HID), dtype=np.float32)
    for c in range(NC):
        o = res.results[c]["out"]          # [NCH, 2*TB, HID]
        for j in range(NCH):
            for b in range(B):
                y[b, j * CH + c * TB:j * CH + (c + 1) * TB, :] = \
                    o[j, b * TB:(b + 1) * TB, :]
    return y


if __name__ == "__main__":
    rng = np.random.default_rng(0)
    sc = 1.0 / np.sqrt(HID)
    inputs = {
        "x": rng.standard_normal((B, L, HID), dtype=np.float32),
        "Wq": rng.standard_normal((HID, HID), dtype=np.float32) * sc,
        "Wk": rng.standard_normal((HID, HID), dtype=np.float32) * sc,
        "Wv": rng.standard_normal((HID, HID), dtype=np.float32) * sc,
        "Wo": rng.standard_normal((HID, HID), dtype=np.float32) * sc,
        "cos": rng.random((L, D), dtype=np.float32),
        "sin": rng.random((L, D), dtype=np.float32),
    }
    y = kernel(**inputs)
    print("ran:", y.shape, y.dtype)


# revision 16
# speedup vs baseline: 1.3398x; 1.3398x over previous
"""Distributed RoPE causal attention for Trainium2 (8 NeuronCores), v2.

Sharding: each core owns 2 heads (tensor-parallel over 16 heads) for both
batches. One 8-core AllToAll per lq-chunk redistributes per-head outputs so
every core gets a 64-token slice (per batch) of ALL heads, then runs 1/8 of
the output projection for that chunk.

v2: software-pipelined emission — the next step's projection matmuls and
RoPE ops are interleaved between attention t-iterations so the PE/DVE FIFOs
always have ready work during the ACT-bound exp chain; RoPE is staged
through one bf16 SBUF copy (2x/4x DVE modes); softmax denominators ride at
partition 0 of the AV psum (direct PSUM reciprocal).
"""
import sys

sys.path.insert(0, "/opt/trn_rl_repo")

import numpy as np
import ml_dtypes

import concourse.bass as bass
import concourse.tile as tile
from concourse import bacc, mybir
from concourse import bass_utils

B, L, H, D = 2, 2048, 16, 64
HID = H * D
NC = 8
CH = 512          # lq chunk width
NCH = L // CH     # 4 chunks per batch
NT = L // 128     # 16 k-tiles of 128 per batch
TB = CH // NC     # 64-token slice each core owns per (chunk, batch)
F32 = mybir.dt.float32
BF16 = mybir.dt.bfloat16
AF = mybir.ActivationFunctionType
ALU = mybir.AluOpType

_CACHE = {}


def build():
    nc = bacc.Bacc("TRN2", target_bir_lowering=False, debug=False, num_devices=NC)

    xT_e = nc.dram_tensor("xT", [B, HID, L], BF16, kind="ExternalInput")
    wq_e = nc.dram_tensor("wq", [HID, 128], BF16, kind="ExternalInput")
    wk_e = nc.dram_tensor("wk", [HID, 128], BF16, kind="ExternalInput")
    wv_e = nc.dram_tensor("wv", [HID, 128], BF16, kind="ExternalInput")
    wo_e = nc.dram_tensor("wo", [HID, HID], BF16, kind="ExternalInput")
    cos_e = nc.dram_tensor("cosd", [128, 2, L], BF16, kind="ExternalInput")
    sin_e = nc.dram_tensor("sind", [128, 2, L], BF16, kind="ExternalInput")
    tri_e = nc.dram_tensor("tri", [128, 128], BF16, kind="ExternalInput")
    out_e = nc.dram_tensor("out", [NCH, 2 * TB, HID], F32, kind="ExternalOutput")

    xT_r = xT_e.ap().rearrange("b (kt p) l -> b p kt l", p=128)

    with tile.TileContext(nc) as tc:
        with tc.tile_pool(name="const", bufs=1) as cpool, \
             tc.tile_pool(name="persist", bufs=1) as ppool, \
             tc.tile_pool(name="xin", bufs=4) as xpool, \
             tc.tile_pool(name="tmp", bufs=4) as tpool, \
             tc.tile_pool(name="ptp", bufs=8) as ptpool, \
             tc.tile_pool(name="osb", bufs=3) as opool, \
             tc.tile_pool(name="fin", bufs=2) as fpool, \
             tc.tile_pool(name="ps_proj", bufs=1, space="PSUM") as ps_proj, \
             tc.tile_pool(name="ps_s", bufs=2, space="PSUM") as ps_s, \
             tc.tile_pool(name="ps_o", bufs=2, space="PSUM") as ps_o, \
             tc.tile_pool(name="dram", bufs=1, space="DRAM") as dpool:

            wq_sb = cpool.tile([128, 8, 128], BF16)
            wk_sb = cpool.tile([128, 8, 128], BF16)
            wv_sb = cpool.tile([128, 8, 128], BF16)
            wo_sb = cpool.tile([128, 8, HID], BF16)
            nc.sync.dma_start(wq_sb[:], wq_e.ap().rearrange("(kt p) m -> p kt m", p=128))
            nc.sync.dma_start(wk_sb[:], wk_e.ap().rearrange("(kt p) m -> p kt m", p=128))
            nc.sync.dma_start(wv_sb[:], wv_e.ap().rearrange("(kt p) m -> p kt m", p=128))
            cosd_sb = cpool.tile([128, 2, L], BF16)
            sind_sb = cpool.tile([128, 2, L], BF16)
            tri2_sb = cpool.tile([128, 2, 128], BF16)
            nc.sync.dma_start(tri2_sb[:, 0], tri_e[:, :])
            nc.sync.dma_start(tri2_sb[:, 1], tri_e[:, :])

            # persistent per-core tensors: q and k fused on one axis so RoPE
            # runs as single [128, 2, CH] DVE ops
            qkT_sb = ppool.tile([128, 2, B, L], BF16)   # [64*hl+d, q|k, b, l]
            v_sb = ppool.tile([128, B, NT, 130], BF16)  # [lk%128, b, t, 65*hl + (ones|d)]

            # chunks 0-2 gather both batches in one A2A; the last chunk goes
            # per batch so most of its output projection hides under the
            # final t-loop
            a2a_in = [dpool.tile([NC, 128, 2 * TB], BF16, name=f"a2ain{i}",
                                 tag=f"a2ain{i}") for i in range(NCH - 1)]
            a2a_out = [dpool.tile([NC, 128, 2 * TB], BF16, name=f"a2aout{i}",
                                  tag=f"a2aout{i}") for i in range(NCH - 1)]
            a2a_in3 = [dpool.tile([NC, 128, TB], BF16, name=f"a2ain3{b}",
                                  tag=f"a2ain3{b}") for b in range(B)]
            a2a_out3 = [dpool.tile([NC, 128, TB], BF16, name=f"a2aout3{b}",
                                   tag=f"a2aout3{b}") for b in range(B)]


            steps = [(j, b) for j in range(NCH) for b in range(B)]

            def emit_loads(i):
                j, b = steps[i]
                ls = j * CH
                xc = xpool.tile([128, 8, CH], BF16, name="xc")
                for kt in range(8):
                    nc.sync.dma_start(xc[:, kt], xT_r[b][:, kt, ls:ls + CH])
                if b == 0:
                    nc.sync.dma_start(cosd_sb[:, :, ls:ls + CH],
                                      cos_e[:, :, ls:ls + CH])
                    nc.sync.dma_start(sind_sb[:, :, ls:ls + CH],
                                      sin_e[:, :, ls:ls + CH])
                if i == 1:
                    nc.sync.dma_start(
                        wo_sb[:], wo_e.ap().rearrange("(kt p) m -> p kt m", p=128))
                return xc

            def proj_closures(i, xc):
                """Fine-grained closures for step i's QKV projections + RoPE,
                to be emitted interleaved into the previous t-loop."""
                j, b = steps[i]
                ls = j * CH
                box = {}

                def c_q():
                    box["pp"] = ps_proj.tile([128, 2, CH], F32, tag="proj", name="pp")
                    for kt in range(8):
                        nc.tensor.matmul(box["pp"][:, 0], wq_sb[:, kt], xc[:, kt],
                                         start=(kt == 0), stop=(kt == 7))

                def c_k():
                    for kt in range(8):
                        nc.tensor.matmul(box["pp"][:, 1], wk_sb[:, kt], xc[:, kt],
                                         start=(kt == 0), stop=(kt == 7))

                def c_stage():
                    # single PSUM reader: everything downstream is bf16 SBUF
                    box["ppb"] = tpool.tile([128, 2, CH], BF16, name="ppb")
                    nc.vector.tensor_copy(box["ppb"][:], box["pp"][:])

                def c_rot():
                    rot = box["rot"] = tpool.tile([128, 2, CH], BF16, name="rot")
                    for seg in range(4):
                        po_, pi_ = 32 * seg, 32 * (seg ^ 1)
                        nc.vector.tensor_copy(rot[po_:po_ + 32], box["ppb"][pi_:pi_ + 32])

                def c_mul():
                    box["t1"] = tpool.tile([128, 2, CH], BF16, name="t1")
                    nc.vector.tensor_mul(box["t1"][:], box["ppb"][:],
                                         cosd_sb[:, :, ls:ls + CH])
                    box["t2"] = tpool.tile([128, 2, CH], BF16, name="t2")
                    nc.vector.tensor_mul(box["t2"][:], box["rot"][:],
                                         sind_sb[:, :, ls:ls + CH])

                def c_add():
                    nc.vector.tensor_add(qkT_sb[:, :, b, ls:ls + CH],
                                         box["t1"][:], box["t2"][:])

                def c_v0():
                    box["pv"] = ps_proj.tile([128, 4, 128], F32, tag="proj", name="pv")
                    for tl in (0, 1):
                        for kt in range(8):
                            nc.tensor.matmul(box["pv"][:, tl],
                                             xc[:, kt, 128 * tl:128 * tl + 128],
                                             wv_sb[:, kt], start=(kt == 0),
                                             stop=(kt == 7), skip_group_check=True)

                def c_v1():
                    for tl in (2, 3):
                        for kt in range(8):
                            nc.tensor.matmul(box["pv"][:, tl],
                                             xc[:, kt, 128 * tl:128 * tl + 128],
                                             wv_sb[:, kt], start=(kt == 0),
                                             stop=(kt == 7), skip_group_check=True)

                def c_vcopy():
                    nc.vector.memset(v_sb[:, b, 4 * j:4 * j + 4, 64::65], 1.0)
                    vv = v_sb.rearrange("p b t (hl e) -> p b t hl e", e=65)
                    nc.vector.tensor_copy(
                        vv[:, b, 4 * j:4 * j + 4, :, 0:64],
                        box["pv"].rearrange("p tl (hl d) -> p tl hl d", d=64))

                return [c_q, c_k, c_stage, c_rot, c_mul, c_add, c_v0, c_v1, c_vcopy]

            def outproj_closures(j):
                """1/8 of chunk j's output projection (this core's 2x64
                tokens, full 1024 out cols)."""
                box = {}

                def c0():
                    box["oj"] = fpool.tile([128, 8, 2 * TB], BF16, name="oj")
                    nc.sync.dma_start(box["oj"][:],
                                      a2a_out[j].rearrange("s p t -> p s t"))
                    box["py"] = ps_proj.tile([128, 2, 512], F32, tag="proj", name="py")
                    for kt in range(8):
                        nc.tensor.matmul(box["py"][:, 0], box["oj"][:, kt],
                                         wo_sb[:, kt, 0:512],
                                         start=(kt == 0), stop=(kt == 7))

                def c1():
                    for kt in range(8):
                        nc.tensor.matmul(box["py"][:, 1], box["oj"][:, kt],
                                         wo_sb[:, kt, 512:1024],
                                         start=(kt == 0), stop=(kt == 7))
                    ysb = fpool.tile([128, 2, 512], F32, name="ysb")
                    nc.vector.tensor_copy(ysb[:], box["py"][:])
                    nc.sync.dma_start(out_e[j], ysb.rearrange("p n m -> p (n m)"))

                return [c0, c1]

            def outproj3_closures(b):
                """Last chunk, one batch: 64-token stationary."""
                box = {}

                def c0():
                    box["oj"] = fpool.tile([128, 8, TB], BF16, name="oj3")
                    nc.sync.dma_start(box["oj"][:],
                                      a2a_out3[b].rearrange("s p t -> p s t"))
                    box["py"] = ps_proj.tile([64, 2, 512], F32, tag="proj", name="py3")
                    for kt in range(8):
                        nc.tensor.matmul(box["py"][:, 0], box["oj"][:, kt],
                                         wo_sb[:, kt, 0:512],
                                         start=(kt == 0), stop=(kt == 7))

                def c1():
                    for kt in range(8):
                        nc.tensor.matmul(box["py"][:, 1], box["oj"][:, kt],
                                         wo_sb[:, kt, 512:1024],
                                         start=(kt == 0), stop=(kt == 7))
                    ysb = fpool.tile([64, 2, 512], F32, name="ysb3")
                    nc.vector.tensor_copy(ysb[:], box["py"][:])
                    nc.sync.dma_start(out_e[NCH - 1, b * TB:(b + 1) * TB, :],
                                      ysb.rearrange("p n m -> p (n m)"))

                return [c0, c1]

            def emit_attention(i, pend):
                """Step i's attention t-loop; `pend` closures are drained
                between t-iterations (after the mask, before the AV matmuls)
                so the PE/DVE FIFOs have dep-free work during the exp wait."""
                j, b = steps[i]
                ls = j * CH
                niter = 4 * j + 4
                n0 = len(pend)
                emitted = 0
                po_t = [ps_o.tile([65, CH], F32, tag="o", name=f"po{hl}")
                        for hl in range(2)]
                for t in range(niter):
                    tl = t - 4 * j
                    o0 = 128 * tl if tl > 0 else 0   # skip fully-masked cols
                    pss = ps_s.tile([128, 2, CH], F32, tag="s", name="pss")
                    for hl in range(2):
                        hp = 64 * hl
                        nc.tensor.matmul(
                            pss[:, hl, o0:CH],
                            qkT_sb[hp:hp + 64, 1, b, 128 * t:128 * t + 128],
                            qkT_sb[hp:hp + 64, 0, b, ls + o0:ls + CH],
                            start=True, stop=True)
                    pt = ptpool.tile([128, 2, CH], BF16, name="pt")
                    nc.scalar.activation(pt[:, :, o0:CH], pss[:, :, o0:CH],
                                         AF.Exp, scale=0.125)
                    if tl >= 0:
                        nc.vector.tensor_mul(pt[:, :, o0:o0 + 128],
                                             pt[:, :, o0:o0 + 128], tri2_sb[:])
                    # drain pipelined closures: spread linearly over the loop
                    target = (t + 1) * n0 // niter
                    while emitted < target:
                        pend[emitted]()
                        emitted += 1
                    for hl in range(2):
                        nc.tensor.matmul(po_t[hl][:, o0:CH],
                                         v_sb[:, b, t, 65 * hl:65 * hl + 65],
                                         pt[:, hl, o0:CH], start=(t == 0),
                                         stop=(t == niter - 1),
                                         skip_group_check=True)
                while emitted < n0:
                    pend[emitted]()
                    emitted += 1
                return po_t

            def emit_epilogue(i, po_t):
                j, b = steps[i]
                # NB: custom-DVE recip misreads PSUM at base partition 64, so
                # stage the denominator rows through SBUF, then one batched
                # reciprocal for both heads.
                dr = tpool.tile([1, 2, CH], F32, name="dr", bufs=2)
                nc.vector.tensor_copy(dr[:, 0], po_t[0][64:65, :])
                nc.vector.tensor_copy(dr[:, 1], po_t[1][64:65, :])
                den = tpool.tile([1, 2, CH], F32, name="den", bufs=2)
                nc.vector.reciprocal_approx_fast(den[:], dr[:])
                rb = tpool.tile([64, 2, CH], F32, name="rb", bufs=2)
                nc.gpsimd.partition_broadcast(rb[:], den[:])
                o_sb = opool.tile([128, CH], BF16, name="o_sb")
                for hl in range(2):
                    nc.vector.tensor_mul(o_sb[64 * hl:64 * hl + 64, :],
                                         po_t[hl][0:64, :], rb[:, hl, :])
                if j == NCH - 1:
                    nc.sync.dma_start(a2a_in3[b].rearrange("s p t -> p s t"),
                                      o_sb.rearrange("p (s t) -> p s t", s=NC))
                    nc.gpsimd.collective_compute(
                        "AllToAll", ALU.bypass,
                        replica_groups=[list(range(NC))],
                        ins=[a2a_in3[b][:].opt()],
                        outs=[a2a_out3[b][:].opt()],
                    )
                else:
                    nc.sync.dma_start(
                        a2a_in[j].rearrange("s p (b t) -> b p s t", b=B)[b],
                        o_sb.rearrange("p (s t) -> p s t", s=NC))
                    if b == B - 1:
                        nc.gpsimd.collective_compute(
                            "AllToAll", ALU.bypass,
                            replica_groups=[list(range(NC))],
                            ins=[a2a_in[j][:].opt()],
                            outs=[a2a_out[j][:].opt()],
                        )

            # ---- software-pipelined emission ----
            xc0 = emit_loads(0)
            for cl in proj_closures(0, xc0):
                cl()
            # outproj(m) drains 2+ steps after a2a(m) fires: the first
            # collective of a NEFF costs 30-100us (ncfw bootstrap) and the
            # chain serializes, so early gathers need a lot of slack.
            due = {(2, 1): [0], (3, 0): [1], (3, 1): [2]}
            for i in range(len(steps)):
                j, b = steps[i]
                pend = []
                if i + 1 < len(steps):
                    xc_n = emit_loads(i + 1)
                    pend += proj_closures(i + 1, xc_n)
                for m in due.get((j, b), ()):
                    pend += outproj_closures(m)
                if (j, b) == (NCH - 1, 1):
                    pend += outproj3_closures(0)
                po_t = emit_attention(i, pend)
                emit_epilogue(i, po_t)
            for cl in outproj3_closures(1):
                cl()

    nc.compile()
    return nc


def _shards(x, Wq, Wk, Wv, Wo, cos, sin):
    bf = ml_dtypes.bfloat16
    xT = np.ascontiguousarray(x.transpose(0, 2, 1)).astype(bf)          # (B, HID, L)
    woT = np.ascontiguousarray(Wo.T).astype(bf)                          # (HID, HID)
    cosT = cos.T.astype(np.float32)                                      # (D, L)
    sinT = sin.T.astype(np.float32)
    cos2 = np.concatenate([cosT, cosT], axis=0)                          # (128, L)
    spm = np.concatenate([-sinT[:32], sinT[32:]], axis=0)                # (64, L)
    sinpm = np.concatenate([spm, spm], axis=0)                           # (128, L)
    cosd = np.ascontiguousarray(np.stack([cos2, cos2], axis=1)).astype(bf)
    sind = np.ascontiguousarray(np.stack([sinpm, sinpm], axis=1)).astype(bf)
    # lower-triangular 128x128 mask for the diagonal band
    p = np.arange(128)[:, None]
    f = np.arange(128)[None, :]
    tri = (p <= f).astype(np.float32).astype(bf)                         # (128, 128)

    in_maps = []
    for c in range(NC):
        rows = slice(2 * c * 64, (2 * c + 2) * 64)
        in_maps.append({
            "xT": xT,
            "wq": np.ascontiguousarray(Wq[rows].T).astype(bf),
            "wk": np.ascontiguousarray(Wk[rows].T).astype(bf),
            "wv": np.ascontiguousarray(Wv[rows].T).astype(bf),
            "wo": woT,
            "cosd": cosd,
            "sind": sind,
            "tri": tri,
        })
    return in_maps


def kernel(x, Wq, Wk, Wv, Wo, cos, sin, trace=False):
    x = np.asarray(x, dtype=np.float32)
    Wq = np.asarray(Wq, dtype=np.float32)
    Wk = np.asarray(Wk, dtype=np.float32)
    Wv = np.asarray(Wv, dtype=np.float32)
    Wo = np.asarray(Wo, dtype=np.float32)
    cos = np.asarray(cos, dtype=np.float32)
    sin = np.asarray(sin, dtype=np.float32)

    if "nc" not in _CACHE:
        _CACHE["nc"] = build()
    nc = _CACHE["nc"]

    in_maps = _shards(x, Wq, Wk, Wv, Wo, cos, sin)
    kw = {}
    if trace and _CACHE.get("trace_all_cores"):
        kw["trace_cores"] = list(range(NC))
    res = bass_utils.run_bass_kernel_spmd(
        nc, in_maps, core_ids=list(range(NC)), trace=trace, **kw)
    _CACHE["last_result"] = res

    y = np.empty((B, L, HID), dtype=np.float32)
    for c in range(NC):
        o = res.results[c]["out"]          # [NCH, 2*TB, HID]
        for j in range(NCH):
            for b in range(B):
                y[b, j * CH + c * TB:j * CH + (c + 1) * TB, :] = \
                    o[j, b * TB:(b + 1) * TB, :]
    return y


if __name__ == "__main__":
    rng = np.random.default_rng(0)
    sc = 1.0 / np.sqrt(HID)
    inputs = {
        "x": rng.standard_normal((B, L, HID), dtype=np.float32),
        "Wq": rng.standard_normal((HID, HID), dtype=np.float32) * sc,
        "Wk": rng.standard_normal((HID, HID), dtype=np.float32) * sc,
        "Wv": rng.standard_normal((HID, HID), dtype=np.float32) * sc,
        "Wo": rng.standard_normal((HID, HID), dtype=np.float32) * sc,
        "cos": rng.random((L, D), dtype=np.float32),
        "sin": rng.random((L, D), dtype=np.float32),
    }
    y = kernel(**inputs)
    print("ran:", y.shape, y.dtype)
